# revision 4
# baseline (speedup 1.0000x reference)
"""Trainium2 kernel for running-average pooling with cached state.

Math (per batch n):
  G[t] = cached_len*cached_avg + cumsum(x[:, n, :], axis=0)[t]
  y[t] = G[t] / (t + 1 + cached_len)
  new_cached_len = cached_len + T ; new_cached_avg = y[T-1]

Sharding: data-parallel over N=16 batches -> 2 batches per core on 8 cores.

Per-core algorithm: blocked cumsum along T via one triangular matmul per
127-row tile per batch. The stationary matrix M1 is triu(ones(128,128)) with
its first column set to all-ones, and the moving tile holds the running carry
row at partition 0 with 127 x-rows at partitions 1..127:
  psum[m] = carry + sum(x_rows[0:m])   (m = 1..127  -> outputs)
  psum[0] = carry + sum(all 127 rows)  (= carry for the next tile)
The carry chains tile-to-tile through an aligned partition-0 ACT copy.
Outputs are scaled by precomputed reciprocals 1/(127*i + m + len) on DVE.

Both batches ride in one SBUF tile ([128, GROUP, 1024], batch n at free
columns n*512..) so every DMA reads/writes a fully contiguous DRAM range
(T-rows of 4KB back to back): one ~2MB load + one ~2MB store per group.
"""

import numpy as np

T, N_FULL_BATCH, C = 4096, 16, 512
NB = 2        # batches per core
CC = NB * C   # elements per T-row in per-core DRAM shard
NCORES = 8
TR = 127      # x-rows per full tile
NFULL = 32    # number of full tiles; last tile has T - TR*NFULL = 32 rows
LASTR = T - TR * NFULL
GROUP = 4     # tiles per DMA group
NG = 8        # full groups (NG * GROUP == NFULL)
NTILES = NFULL + 1

_cached_nc = None


def _build():
    from contextlib import ExitStack

    import concourse.bass as bass
    import concourse.bacc as bacc
    import concourse.tile as tile
    from concourse import mybir

    f32 = mybir.dt.float32
    i32 = mybir.dt.int32

    nc = bacc.Bacc(None, target_bir_lowering=False)
    x_h = nc.declare_dram_parameter("x", [T, NB, C], f32, isOutput=False)
    len_h = nc.declare_dram_parameter("cached_len", [NB], i32, isOutput=False)
    avg_h = nc.declare_dram_parameter("cached_avg", [NB, C], f32, isOutput=False)
    y_h = nc.declare_dram_parameter("y", [T, NB, C], f32, isOutput=True)

    m1_np = np.triu(np.ones((128, 128), dtype=np.float32))
    m1_np[:, 0] = 1.0
    grid_np = (
        np.arange(NTILES, dtype=np.float32)[None, :] * TR
        + np.arange(128, dtype=np.float32)[:, None]
    )
    grid_np[0, :] = 1.0  # row 0 is never an output; avoid 1/0
    m1_d = nc.inline_tensor(m1_np, name="m1c")
    grid_d = nc.inline_tensor(grid_np, name="gridc")

    ROWS_G = TR * GROUP  # 508 T-rows per group

    def group_ap(h, g):
        full = h[:]
        return bass.AP(
            tensor=full.tensor,
            offset=ROWS_G * g * CC,
            ap=[[CC, TR], [TR * CC, GROUP], [1, CC]],
        )

    with ExitStack() as ctx:
        tc = ctx.enter_context(tile.TileContext(nc))
        sing = ctx.enter_context(tc.tile_pool(name="sing", bufs=1))
        xp = ctx.enter_context(tc.tile_pool(name="xp", bufs=3))
        yp = ctx.enter_context(tc.tile_pool(name="yp", bufs=3))
        psp = ctx.enter_context(tc.tile_pool(name="psp", bufs=6, space="PSUM"))

        m1 = sing.tile([128, 128], f32, name="m1")
        nc.sync.dma_start(out=m1[:], in_=m1_d[:])
        grid = sing.tile([128, NTILES], f32, name="grid")
        nc.sync.dma_start(out=grid[:], in_=grid_d[:])

        len_i = sing.tile([128, NB], i32, name="len_i")
        lsrc = len_h[:]
        nc.sync.dma_start(
            out=len_i[:],
            in_=bass.AP(tensor=lsrc.tensor, offset=0, ap=[[0, 128], [1, NB]]),
        )
        len_f = sing.tile([128, NB], f32, name="len_f")
        nc.vector.tensor_copy(len_f[:], len_i[:])

        counts = sing.tile([128, NB, NTILES], f32, name="counts")
        recip = sing.tile([128, NB, NTILES], f32, name="recip")
        for n in range(NB):
            nc.vector.tensor_scalar_add(counts[:, n, :], grid[:], len_f[:, n : n + 1])
        nc.vector.reciprocal(recip[:], counts[:])

        # group-0 buffer + base carries = cached_avg * len at partition 0
        xb_cur = xp.tile([128, GROUP, CC], f32, name="xb")
        nc.sync.dma_start(out=xb_cur[0:1, 0, :], in_=avg_h[:])
        for n in range(NB):
            nc.vector.tensor_scalar_mul(
                xb_cur[0:1, 0, n * C : (n + 1) * C],
                xb_cur[0:1, 0, n * C : (n + 1) * C],
                len_f[0:1, n : n + 1],
            )

        for g in range(NG):
            nc.sync.dma_start(out=xb_cur[1:128, :, :], in_=group_ap(x_h, g))

            if g + 1 < NG:
                xb_nxt = xp.tile([128, GROUP, CC], f32, name="xb")
            else:
                xb_nxt = xp.tile([33, CC], f32, name="xbl", bufs=1)
            yb = yp.tile([128, GROUP, CC], f32, name="yb")

            for j in range(GROUP):
                i = GROUP * g + j
                for n in range(NB):
                    ps = psp.tile([128, C], f32, name="ps")
                    nc.tensor.matmul(
                        out=ps[:],
                        lhsT=m1[:],
                        rhs=xb_cur[:, j, n * C : (n + 1) * C],
                        start=True,
                        stop=True,
                    )
                    if j + 1 < GROUP:
                        tgt = xb_cur[0:1, j + 1, n * C : (n + 1) * C]
                    elif g + 1 < NG:
                        tgt = xb_nxt[0:1, 0, n * C : (n + 1) * C]
                    else:
                        tgt = xb_nxt[0:1, n * C : (n + 1) * C]
                    nc.scalar.activation(
                        out=tgt, in_=ps[0:1, :], func=mybir.ActivationFunctionType.Copy
                    )
                    nc.vector.tensor_scalar_mul(
                        yb[:, j, n * C : (n + 1) * C], ps[:], recip[:, n, i : i + 1]
                    )

            nc.gpsimd.dma_start(out=group_ap(y_h, g), in_=yb[1:128, :, :])
            xb_cur = xb_nxt

        # last (short) tile: 32 rows, carries already at xb_cur[0:1, :]
        nc.sync.dma_start(out=xb_cur[1 : 1 + LASTR, :], in_=x_h[TR * NFULL : T])
        ybl = yp.tile([33, CC], f32, name="ybl", bufs=1)
        for n in range(NB):
            ps = psp.tile([128, C], f32, name="ps")
            nc.tensor.matmul(
                out=ps[0 : 1 + LASTR, :],
                lhsT=m1[0 : 1 + LASTR, 0 : 1 + LASTR],
                rhs=xb_cur[0 : 1 + LASTR, n * C : (n + 1) * C],
                start=True,
                stop=True,
            )
            nc.vector.tensor_scalar_mul(
                ybl[0 : 1 + LASTR, n * C : (n + 1) * C],
                ps[0 : 1 + LASTR, :],
                recip[0 : 1 + LASTR, n, NFULL : NFULL + 1],
            )
        nc.gpsimd.dma_start(out=y_h[TR * NFULL : T], in_=ybl[1 : 1 + LASTR, :])

    nc.finalize()
    return nc


def _get_nc():
    global _cached_nc
    if _cached_nc is None:
        _cached_nc = _build()
    return _cached_nc


def kernel(x, cached_len, cached_avg, _trace=False):
    from concourse.bass_utils import run_bass_kernel_spmd

    x = np.asarray(x, dtype=np.float32)
    cached_len = np.asarray(cached_len, dtype=np.int32)
    cached_avg = np.asarray(cached_avg, dtype=np.float32)

    nc = _get_nc()
    in_maps = []
    for c in range(NCORES):
        lo, hi = NB * c, NB * (c + 1)
        in_maps.append(
            {
                "x": np.ascontiguousarray(x[:, lo:hi, :]),
                "cached_len": np.ascontiguousarray(cached_len[lo:hi]),
                "cached_avg": np.ascontiguousarray(cached_avg[lo:hi, :]),
            }
        )
    res = run_bass_kernel_spmd(nc, in_maps, core_ids=list(range(NCORES)), trace=_trace)
    new_x = np.concatenate([res.results[c]["y"] for c in range(NCORES)], axis=1)
    new_cached_len = cached_len + T
    new_cached_avg = new_x[-1].copy()
    if _trace:
        return (new_x, new_cached_len, new_cached_avg), res
    return new_x, new_cached_len, new_cached_avg


# revision 6
# speedup vs baseline: 1.1200x; 1.1200x over previous
"""Trainium2 kernel for running-average pooling with cached state.

Math (per batch n):
  G[t] = cached_len*cached_avg + cumsum(x[:, n, :], axis=0)[t]
  y[t] = G[t] / (t + 1 + cached_len)
  new_cached_len = cached_len + T ; new_cached_avg = y[T-1]

Sharding: data-parallel over N=16 batches -> 2 batches per core on 8 cores.

Per-core algorithm: blocked cumsum along T via one triangular matmul per
127-row tile per batch. The stationary matrix M1 is triu(ones(128,128)) with
its first column set to all-ones, and the moving tile holds the running carry
row at partition 0 with 127 x-rows at partitions 1..127:
  psum[m] = carry + sum(x_rows[0:m])   (m = 1..127  -> outputs)
  psum[0] = carry + sum(all 127 rows)  (= carry for the next tile)
The carry chains tile-to-tile through an aligned partition-0 ACT copy.
Outputs are scaled by precomputed reciprocals 1/(127*i + m + len) on DVE.

Both batches ride in one SBUF tile ([128, GROUP, 1024], batch n at free
columns n*512..) so every DMA reads/writes a fully contiguous DRAM range
(T-rows of 4KB back to back): one ~2MB load + one ~2MB store per group.
"""

import numpy as np

T, N_FULL_BATCH, C = 4096, 16, 512
NB = 2        # batches per core
CC = NB * C   # elements per T-row in per-core DRAM shard
NCORES = 8
TR = 127      # x-rows per full tile
NFULL = 32    # number of full tiles; last tile has T - TR*NFULL = 32 rows
LASTR = T - TR * NFULL
GROUP = 4     # tiles per DMA group
NG = 8        # full groups (NG * GROUP == NFULL)
NTILES = NFULL + 1

_cached_nc = None


def _build():
    from contextlib import ExitStack

    import concourse.bass as bass
    import concourse.bacc as bacc
    import concourse.tile as tile
    from concourse import mybir

    f32 = mybir.dt.float32
    i32 = mybir.dt.int32

    nc = bacc.Bacc(None, target_bir_lowering=False)
    x_h = nc.declare_dram_parameter("x", [T, NB, C], f32, isOutput=False)
    len_h = nc.declare_dram_parameter("cached_len", [NB], i32, isOutput=False)
    avg_h = nc.declare_dram_parameter("cached_avg", [NB, C], f32, isOutput=False)
    y_h = nc.declare_dram_parameter("y", [T, NB, C], f32, isOutput=True)

    m1_np = np.triu(np.ones((128, 128), dtype=np.float32))
    m1_np[:, 0] = 1.0
    grid_np = (
        np.arange(NTILES, dtype=np.float32)[None, :] * TR
        + np.arange(128, dtype=np.float32)[:, None]
    )
    grid_np[0, :] = 1.0  # row 0 is never an output; avoid 1/0
    m1_d = nc.inline_tensor(m1_np, name="m1c")
    grid_d = nc.inline_tensor(grid_np, name="gridc")

    ROWS_G = TR * GROUP  # 508 T-rows per group

    def group_ap(h, g):
        full = h[:]
        return bass.AP(
            tensor=full.tensor,
            offset=ROWS_G * g * CC,
            ap=[[CC, TR], [TR * CC, GROUP], [1, CC]],
        )

    with ExitStack() as ctx:
        tc = ctx.enter_context(tile.TileContext(nc))
        sing = ctx.enter_context(tc.tile_pool(name="sing", bufs=1))
        xp = ctx.enter_context(tc.tile_pool(name="xp", bufs=3))
        yp = ctx.enter_context(tc.tile_pool(name="yp", bufs=3))
        psp = ctx.enter_context(tc.tile_pool(name="psp", bufs=6, space="PSUM"))

        m1 = sing.tile([128, 128], f32, name="m1")
        nc.sync.dma_start(out=m1[:], in_=m1_d[:])
        grid = sing.tile([128, NTILES], f32, name="grid")
        nc.sync.dma_start(out=grid[:], in_=grid_d[:])

        len_i = sing.tile([128, NB], i32, name="len_i")
        lsrc = len_h[:]
        nc.sync.dma_start(
            out=len_i[:],
            in_=bass.AP(tensor=lsrc.tensor, offset=0, ap=[[0, 128], [1, NB]]),
        )
        len_f = sing.tile([128, NB], f32, name="len_f")
        nc.vector.tensor_copy(len_f[:], len_i[:])

        counts = sing.tile([128, NB, NTILES], f32, name="counts")
        recip = sing.tile([128, NB, NTILES], f32, name="recip")
        for n in range(NB):
            nc.vector.tensor_scalar_add(counts[:, n, :], grid[:], len_f[:, n : n + 1])
        nc.vector.reciprocal(recip[:], counts[:])

        # group-0 buffer + base carries = cached_avg * len at partition 0
        xb_cur = xp.tile([128, GROUP, CC], f32, name="xb")
        nc.sync.dma_start(out=xb_cur[0:1, 0, :], in_=avg_h[:])
        for n in range(NB):
            nc.vector.tensor_scalar_mul(
                xb_cur[0:1, 0, n * C : (n + 1) * C],
                xb_cur[0:1, 0, n * C : (n + 1) * C],
                len_f[0:1, n : n + 1],
            )

        for g in range(NG):
            nc.gpsimd.dma_start(out=xb_cur[1:128, :, :], in_=group_ap(x_h, g))

            if g + 1 < NG:
                xb_nxt = xp.tile([128, GROUP, CC], f32, name="xb")
            else:
                xb_nxt = xp.tile([33, CC], f32, name="xbl", bufs=1)
            yb = yp.tile([128, GROUP, CC], f32, name="yb")

            for j in range(GROUP):
                i = GROUP * g + j
                for n in range(NB):
                    ps = psp.tile([128, C], f32, name="ps")
                    nc.tensor.matmul(
                        out=ps[:],
                        lhsT=m1[:],
                        rhs=xb_cur[:, j, n * C : (n + 1) * C],
                        start=True,
                        stop=True,
                    )
                    if j + 1 < GROUP:
                        tgt = xb_cur[0:1, j + 1, n * C : (n + 1) * C]
                    elif g + 1 < NG:
                        tgt = xb_nxt[0:1, 0, n * C : (n + 1) * C]
                    else:
                        tgt = xb_nxt[0:1, n * C : (n + 1) * C]
                    nc.scalar.activation(
                        out=tgt, in_=ps[0:1, :], func=mybir.ActivationFunctionType.Copy
                    )
                    nc.vector.tensor_scalar_mul(
                        yb[:, j, n * C : (n + 1) * C], ps[:], recip[:, n, i : i + 1]
                    )

            nc.gpsimd.dma_start(out=group_ap(y_h, g), in_=yb[1:128, :, :])
            xb_cur = xb_nxt

        # last (short) tile: 32 rows, carries already at xb_cur[0:1, :]
        nc.gpsimd.dma_start(out=xb_cur[1 : 1 + LASTR, :], in_=x_h[TR * NFULL : T])
        ybl = yp.tile([33, CC], f32, name="ybl", bufs=1)
        for n in range(NB):
            ps = psp.tile([128, C], f32, name="ps")
            nc.tensor.matmul(
                out=ps[0 : 1 + LASTR, :],
                lhsT=m1[0 : 1 + LASTR, 0 : 1 + LASTR],
                rhs=xb_cur[0 : 1 + LASTR, n * C : (n + 1) * C],
                start=True,
                stop=True,
            )
            nc.vector.tensor_scalar_mul(
                ybl[0 : 1 + LASTR, n * C : (n + 1) * C],
                ps[0 : 1 + LASTR, :],
                recip[0 : 1 + LASTR, n, NFULL : NFULL + 1],
            )
        nc.gpsimd.dma_start(out=y_h[TR * NFULL : T], in_=ybl[1 : 1 + LASTR, :])

    nc.finalize()
    return nc


def _get_nc():
    global _cached_nc
    if _cached_nc is None:
        _cached_nc = _build()
    return _cached_nc


def kernel(x, cached_len, cached_avg, _trace=False):
    from concourse.bass_utils import run_bass_kernel_spmd

    x = np.asarray(x, dtype=np.float32)
    cached_len = np.asarray(cached_len, dtype=np.int32)
    cached_avg = np.asarray(cached_avg, dtype=np.float32)

    nc = _get_nc()
    in_maps = []
    for c in range(NCORES):
        lo, hi = NB * c, NB * (c + 1)
        in_maps.append(
            {
                "x": np.ascontiguousarray(x[:, lo:hi, :]),
                "cached_len": np.ascontiguousarray(cached_len[lo:hi]),
                "cached_avg": np.ascontiguousarray(cached_avg[lo:hi, :]),
            }
        )
    res = run_bass_kernel_spmd(nc, in_maps, core_ids=list(range(NCORES)), trace=_trace)
    new_x = np.concatenate([res.results[c]["y"] for c in range(NCORES)], axis=1)
    new_cached_len = cached_len + T
    new_cached_avg = new_x[-1].copy()
    if _trace:
        return (new_x, new_cached_len, new_cached_avg), res
    return new_x, new_cached_len, new_cached_avg
